# revision 1
# baseline (speedup 1.0000x reference)
"""Tensor-parallel GQA attention (Llama-3-8B shape, prefill, start_pos=0) on 8
Trainium2 NeuronCores.

Sharding: core i owns kv-head i and q-heads 4i..4i+3 — wq/wk/wv column-shards,
wo row-shard, x replicated.  Each core computes a partial [2048, 4096] output
(its heads pushed through its wo rows); the host sums the 8 partials
(all-reduce equivalent).

Per-core kernel layout choices (all matmuls N=512, fp32r operands):
  - xT [D, S] on device; projections computed with weights as the stationary
    operand, giving qT/kT/vT in [head_dim, seq] layout directly.
  - RoPE applied in [head_dim, seq] layout; the rotate-half partition swap is
    two SBUF->SBUF DMAs; sin tables are sign-folded on the host, and the
    1/sqrt(hd) score scale is folded into q's sin/cos tables.
  - Scores computed transposed, ST[j, i] = kT.T @ qT, so exp needs no
    transposes and PV consumes Pexp^T directly (lhsT = v tile [j, d],
    rhs = Pexp^T [j, i] -> outT [d, i] accumulated over j-tiles in PSUM).
  - No max-subtraction in softmax (scores bounded, |S| ~ 10); row sums come
    from an all-ones stationary matmul accumulated alongside PV (broadcast
    across partitions), so normalization is one reciprocal + one multiply,
    fused with the PV eviction.  outT overwrites qT storage (dead by then).
  - Causal masking: only j-tiles with j <= chunk max are computed; the 4
    diagonal tiles per (head, i-chunk) get affine_select(fill=0) after exp.
  - wo matmul with lhsT = normalized outT slices, accumulated over the 4
    heads in PSUM; eviction is a plain copy, DMA'd to the partial output.
"""

import math
from contextlib import ExitStack

import numpy as np

import concourse.bass as bass
import concourse.tile as tile
from concourse import bacc, mybir
from concourse.bass_utils import run_bass_kernel_spmd

# ---- problem shape (hardcoded per contract) ----
S = 2048           # seq len
D = 4096           # model dim
HD = 128           # head dim
N_CORES = 8
NQH = 4            # q heads per core
QCOLS = NQH * HD   # 512 wq columns per core
SC_N = 4           # seq chunks of 512
KT_N = D // 128    # 32 contraction tiles
JT_N = S // 128    # 16 key tiles
ECH_N = D // 512   # 8 output column chunks

F32 = mybir.dt.float32
F32R = mybir.dt.float32r
BF16 = mybir.dt.bfloat16

MM_DT = F32R          # matmul operand dtype: F32R (accurate) or BF16 (fast)

_BUILD_CACHE: dict = {}


def _rope_apply(nc, pools, dst_slice, ps, cos_t, sin_t):
    """dst = ps * cos + swap64(ps) * sin   (all [128, 512])."""
    qc = pools["rope_tmp"].tile([128, 512], F32, tag="rope_qc", name="rqc")
    nc.vector.tensor_copy(qc[:], ps)
    qs = pools["rope_tmp"].tile([128, 512], F32, tag="rope_qs", name="rqs")
    nc.sync.dma_start(qs[0:64, :], qc[64:128, :])
    nc.sync.dma_start(qs[64:128, :], qc[0:64, :])
    # cos product straight from PSUM (parallel with the swap DMAs)
    tc_ = pools["rope_tmp"].tile([128, 512], F32, tag="rope_tc", name="rtc")
    nc.vector.tensor_mul(tc_[:], ps, cos_t)
    nc.vector.tensor_mul(qs[:], qs[:], sin_t)
    nc.vector.tensor_add(dst_slice, tc_[:], qs[:])


STAGES = "ABC"  # debug knob: subset of stages to emit


def _emit_body(nc, tc, dram, out, causal: bool):
    with ExitStack() as ctx:
        def pool(name, bufs, space="SBUF"):
            return ctx.enter_context(tc.tile_pool(name=name, bufs=bufs, space=space))

        pools = {
            "w": pool("w", 1),
            "x": pool("x", 2),
            "rope_in": pool("rope_in", 1),
            "rope_tmp": pool("rope_tmp", 2),
            "persist": pool("persist", 1),
            "pexp": pool("pexp", 3),
            "recip": pool("recip", 1),
            "wo": pool("wo", 2),
            "outsb": pool("outsb", 2),
        }

        # resident weights (wq 64KB/part + wk/wv 16KB each), contiguous loads
        wq_sb = pools["w"].tile([128, KT_N, QCOLS], MM_DT, tag="wq", name="wq_sb")
        for wch in range(4):
            nc.sync.dma_start(
                wq_sb[:, wch * 8:(wch + 1) * 8, :],
                dram["wq"][:, wch * 8:(wch + 1) * 8, :],
            )
        wk_sb = pools["w"].tile([128, KT_N, HD], MM_DT, tag="wk", name="wk_sb")
        nc.sync.dma_start(wk_sb[:], dram["wk"][:])
        wv_sb = pools["w"].tile([128, KT_N, HD], MM_DT, tag="wv", name="wv_sb")
        nc.sync.dma_start(wv_sb[:], dram["wv"][:])

        # persistent activations
        kT_sb = pools["persist"].tile([128, S], MM_DT, tag="kT", name="kT_sb")
        # qT doubles as outT: B(h, ic) consumes qT[h, ic] then writes the
        # normalized attention output into the same slice.
        qT_sb = pools["persist"].tile([128, NQH, S], MM_DT, tag="qT", name="qT_sb")
        v_sb = pools["persist"].tile([128, JT_N, HD], MM_DT, tag="v", name="v_sb")
        ones_sb = pools["persist"].tile([128, 128], MM_DT, tag="ones", name="ones_sb")
        nc.sync.dma_start(ones_sb[:], dram["ones"][:])
        ident_sb = pools["persist"].tile([128, 128], MM_DT, tag="ident", name="ident_sb")
        nc.sync.dma_start(ident_sb[:], dram["ident"][:])

        # ---- stage A: projections + RoPE, per 512-wide seq chunk ----
        with tc.tile_pool(name="psA", bufs=1, space="PSUM") as psA:
            for sc in range(SC_N):
                ssl = slice(sc * 512, (sc + 1) * 512)
                rp = pools["rope_in"].tile([HD, 4, 512], F32, tag="rp", name="rp")
                nc.sync.dma_start(rp[:], dram["rope"][sc])
                cq, sq, ck, sk = rp[:, 0, :], rp[:, 1, :], rp[:, 2, :], rp[:, 3, :]

                ps_q = [
                    psA.tile([128, 512], F32, tag=f"psq{h}", name=f"psq{h}")
                    for h in range(NQH)
                ]
                ps_k = psA.tile([128, 512], F32, tag="psk", name="psk")
                ps_vt = psA.tile([128, 512], F32, tag="psvt", name="psvt")
                for ktc in range(KT_N // 2):
                    # batched x stream: 2 contraction tiles per DMA (512 KB)
                    xt = pools["x"].tile([128, 2, 512], MM_DT, tag="xt", name="xt")
                    nc.sync.dma_start(xt[:], dram["xn"][sc, ktc])
                    for ki in range(2):
                        kt = ktc * 2 + ki
                        first, last = kt == 0, kt == KT_N - 1
                        for h in range(NQH):
                            nc.tensor.matmul(
                                ps_q[h][:],
                                wq_sb[:, kt, h * 128:(h + 1) * 128],
                                xt[:, ki, :],
                                start=first,
                                stop=last,
                            )
                        nc.tensor.matmul(
                            ps_k[:], wk_sb[:, kt, :], xt[:, ki, :],
                            start=first, stop=last,
                        )
                        nc.tensor.matmul(
                            ps_vt[:], wv_sb[:, kt, :], xt[:, ki, :],
                            start=first, stop=last,
                        )

                _rope_apply(nc, pools, kT_sb[:, ssl], ps_k[:], ck, sk)
                # v: evict vT then transpose 128x128 blocks to [j, d]
                vt_f = pools["rope_tmp"].tile(
                    [128, 512], MM_DT, tag="vt_f", name="vt_f", bufs=1
                )
                nc.vector.tensor_copy(vt_f[:], ps_vt[:])
                for vi in range(4):
                    ptr = psA.tile([128, 128], MM_DT, tag="pstr", name="pstr")
                    nc.tensor.transpose(
                        ptr[:], vt_f[:, vi * 128:(vi + 1) * 128], ident_sb[:]
                    )
                    nc.vector.tensor_copy(v_sb[:, sc * 4 + vi, :], ptr[:])
                for h in range(NQH):
                    _rope_apply(
                        nc, pools, qT_sb[:, h, ssl], ps_q[h][:], cq, sq
                    )

        if "B" not in STAGES:
            # A-only debug: flush qT so the program has a live output
            dbg = pools["outsb"].tile([128, 2, 512], F32, tag="ob", name="dbg")
            nc.vector.tensor_copy(dbg[:, 0, :], qT_sb[:, 0, 0:512])
            nc.sync.dma_start(out[0, 0], dbg[:])
            return
        # ---- stage B: attention (transposed scores), per (i-chunk, head) ----
        with tc.tile_pool(name="psB", bufs=1, space="PSUM") as psB:
            for ic in range(SC_N):
                isl = slice(ic * 512, (ic + 1) * 512)
                njt = 4 * (ic + 1) if causal else JT_N
                for h in range(NQH):
                    pv = psB.tile([128, 512], F32, tag="pspv", name="pspv")
                    rs = psB.tile([128, 512], F32, tag="psrs", name="psrs")
                    for jt in range(njt):
                        st = psB.tile(
                            [128, 512], F32, tag="psst", name="psst", bufs=3
                        )
                        nc.tensor.matmul(
                            st[:],
                            kT_sb[:, jt * 128:(jt + 1) * 128],
                            qT_sb[:, h, isl],
                            start=True,
                            stop=True,
                        )
                        pe = pools["pexp"].tile([128, 512], MM_DT, tag="pe", name="pe")
                        nc.scalar.activation(
                            pe[:], st[:], mybir.ActivationFunctionType.Exp
                        )
                        if causal and jt >= 4 * ic:
                            nc.gpsimd.affine_select(
                                out=pe[:],
                                in_=pe[:],
                                pattern=[[1, 512]],
                                compare_op=mybir.AluOpType.is_ge,
                                fill=0.0,
                                base=512 * ic - 128 * jt,
                                channel_multiplier=-1,
                            )
                        first, last = jt == 0, jt == njt - 1
                        nc.tensor.matmul(
                            pv[:], v_sb[:, jt, :], pe[:], start=first, stop=last
                        )
                        nc.tensor.matmul(
                            rs[:], ones_sb[:], pe[:], start=first, stop=last
                        )
                    rc = pools["recip"].tile([128, 512], F32, tag="rc", name="rc")
                    nc.vector.reciprocal(rc[:], rs[:])
                    nc.vector.tensor_mul(qT_sb[:, h, isl], pv[:], rc[:])

            if "C" not in STAGES:
                dbg = pools["outsb"].tile([128, 2, 512], F32, tag="ob", name="dbg")
                nc.vector.tensor_copy(dbg[:, 0, :], qT_sb[:, 0, 0:512])
                nc.sync.dma_start(out[0, 0], dbg[:])
                return
            # ---- stage C: wo matmul (outT lives in qT_sb) ----
            for ech in range(ECH_N):
                esl = slice(ech * 512, (ech + 1) * 512)
                woc = pools["wo"].tile([128, NQH, 512], MM_DT, tag="woc", name="woc")
                nc.sync.dma_start(woc[:], dram["wo"][ech])
                for itp in range(JT_N // 2):
                    # pack 2 row-tiles per output DMA (halves DMA issue count)
                    ob = pools["outsb"].tile([128, 2, 512], F32, tag="ob", name="ob")
                    for ii in range(2):
                        it = itp * 2 + ii
                        pc = psB.tile([128, 512], F32, tag="psc", name="psc", bufs=3)
                        for h in range(NQH):
                            nc.tensor.matmul(
                                pc[:],
                                qT_sb[:, h, it * 128:(it + 1) * 128],
                                woc[:, h, :],
                                start=h == 0,
                                stop=h == NQH - 1,
                            )
                        nc.vector.tensor_copy(ob[:, ii, :], pc[:])
                    # alternate the two HWDGE issue queues (sync / scalar)
                    eng = nc.scalar if itp % 2 else nc.sync
                    eng.dma_start(out[ech, itp], ob[:])

def build_nc(causal: bool = True, reps: int = 1):
    nc = bacc.Bacc(
        "TRN2", target_bir_lowering=False, debug=False, num_devices=N_CORES
    )
    dram = {}
    for name, shape, dt in [
        # host-prepermuted layouts: every DMA reads/writes contiguous
        # per-partition runs
        ("xn", [SC_N, KT_N // 2, 128, 2, 512], MM_DT),
        ("wq", [128, KT_N, QCOLS], MM_DT),
        ("wk", [128, KT_N, HD], MM_DT),
        ("wv", [128, KT_N, HD], MM_DT),
        ("wo", [ECH_N, 128, NQH, 512], MM_DT),
        ("rope", [SC_N, HD, 4, 512], F32),
        ("ones", [128, 128], MM_DT),
        ("ident", [128, 128], MM_DT),
    ]:
        dram[name] = nc.dram_tensor(name, shape, dt, kind="ExternalInput").ap()
    out = nc.dram_tensor("out", [ECH_N, JT_N // 2, 128, 2, 512], F32,
                         kind="ExternalOutput").ap()

    with tile.TileContext(nc) as tc:
        for _ in range(reps):
            _emit_body(nc, tc, dram, out, causal)

    nc.compile()
    return nc


def get_nc(causal: bool = True):
    if causal not in _BUILD_CACHE:
        _BUILD_CACHE[causal] = build_nc(causal)
    return _BUILD_CACHE[causal]


def _mm_np(a):
    return np.ascontiguousarray(a).astype(mybir.dt.np(MM_DT))


def prep_in_maps(x, sincos, wq, wk, wv, wo):
    """Host-side shard + layout prep. Returns list of per-core input dicts.

    All tensors are pre-permuted so that every device DMA moves contiguous
    per-partition runs (device DMA engines are far more efficient that way).
    """
    x = np.asarray(x, np.float32)
    assert x.shape == (1, S, D)
    # xn[sc, ktc, p, ki, n] = x[sc*512 + n, (ktc*2 + ki)*128 + p]
    xn = _mm_np(
        x[0].reshape(SC_N, 512, KT_N // 2, 2, 128).transpose(0, 2, 4, 3, 1)
    )

    sincos = np.asarray(sincos, np.float32)
    sin = sincos[:S, :HD]
    cos = sincos[:S, HD:]
    sinT = np.ascontiguousarray(sin.T)
    cosT = np.ascontiguousarray(cos.T)
    sin_sgn = sinT.copy()
    sin_sgn[:64] = -sinT[:64]
    scale = np.float32(1.0 / math.sqrt(HD))
    # rope[sc, d, tbl, n], tbl order: cosq, sinq, cosk, sink
    rope = np.stack(
        [cosT * scale, sin_sgn * scale, cosT, sin_sgn], axis=0
    ).reshape(4, HD, SC_N, 512).transpose(2, 1, 0, 3)
    rope = np.ascontiguousarray(rope)

    wq = np.asarray(wq, np.float32)
    wk = np.asarray(wk, np.float32)
    wv = np.asarray(wv, np.float32)
    wo = np.asarray(wo, np.float32)

    in_maps = []
    for c in range(N_CORES):
        wq_c = wq[:, c * QCOLS:(c + 1) * QCOLS]          # [D, 512]
        wk_c = wk[:, c * HD:(c + 1) * HD]                # [D, 128]
        wv_c = wv[:, c * HD:(c + 1) * HD]
        wo_c = wo[c * QCOLS:(c + 1) * QCOLS, :]          # [512, D]
        in_maps.append(
            {
                "xn": xn,
                # wq[p, kt, m] = wq_c[kt*128 + p, m]
                "wq": _mm_np(
                    wq_c.reshape(KT_N, 128, QCOLS).transpose(1, 0, 2)
                ),
                "wk": _mm_np(
                    wk_c.reshape(KT_N, 128, HD).transpose(1, 0, 2)
                ),
                "wv": _mm_np(
                    wv_c.reshape(KT_N, 128, HD).transpose(1, 0, 2)
                ),
                # wo[ech, p, a, n] = wo_c[a*128 + p, ech*512 + n]
                "wo": _mm_np(
                    wo_c.reshape(NQH, 128, ECH_N, 512).transpose(2, 1, 0, 3)
                ),
                "rope": rope,
                "ones": _mm_np(np.ones((128, 128), np.float32)),
                "ident": _mm_np(np.eye(128, dtype=np.float32)),
            }
        )
    return in_maps


def unpermute_out(out_n):
    """out_n [ech, itp, p, ii, n] -> out [S, D]."""
    return np.ascontiguousarray(
        out_n.transpose(1, 3, 2, 0, 4).reshape(S, D)
    )


def check_mask(full_causal_mask, start_pos) -> bool:
    """Returns True for causal (tril) mask, False for all-allowed."""
    sp = int(start_pos)
    assert sp == 0, f"kernel specialized for start_pos=0, got {sp}"
    m = np.asarray(full_causal_mask)
    assert m.shape == (1, 1, S, S)
    m = m[0, 0]
    tril = np.tril(np.ones((S, S), dtype=bool))
    if (m == tril).all():
        return True
    if m.all():
        return False
    raise AssertionError("unsupported mask pattern")


def kernel(
    x,
    start_pos,
    sincos,
    full_causal_mask,
    wq,
    wk,
    wv,
    wo,
    cache_k,
    cache_v,
):
    causal = check_mask(full_causal_mask, start_pos)
    # cache_k/cache_v are zero and fully overwritten in the attended region
    # (start_pos=0, seq_len == max_seq_len) — they do not affect the output.
    nc = get_nc(causal)
    in_maps = prep_in_maps(x, sincos, wq, wk, wv, wo)
    res = run_bass_kernel_spmd(nc, in_maps, list(range(N_CORES)))
    acc = res.results[0]["out"].astype(np.float32)
    for c in range(1, N_CORES):
        acc = acc + res.results[c]["out"]
    return unpermute_out(acc)[np.newaxis]



# revision 7
# speedup vs baseline: 3.4394x; 3.4394x over previous
"""Tensor-parallel GQA attention (Llama-3-8B shape, prefill, start_pos=0) on 8
Trainium2 NeuronCores.

Sharding: core i owns kv-head i and q-heads 4i..4i+3 — wq/wk/wv column-shards,
wo row-shard, x replicated.  Each core computes a partial [2048, 4096] output
(its heads pushed through its wo rows); the host sums the 8 partials
(all-reduce equivalent).

v2 layout (all matmul operands bf16, fp32 PSUM accumulate):
  - Stages are interleaved per 512-wide seq chunk: A(sc) projections+RoPE,
    B(ic) attention, C(ic) wo-matmul, emitted A0 A1 B0 C0 A2 B1 C1 A3 B2 C2
    B3 C3 so the tensor engine never drains between stages.
  - Stage A is out-tile-major: x for a whole chunk is SBUF-resident
    ([128, 32, 512] bf16, double-buffered), each of the 6 output tiles
    (k, q0..q3, vT) accumulates its 32 contraction matmuls in a single PSUM
    bank, so stage A holds only 2 PSUM banks and B/C can run concurrently.
  - RoPE: all 5 rotated tiles (k, q0..3) evict into one [128, 5, 512] f32
    SBUF tile (freeing PSUM immediately); rotate-half is one pair of
    SBUF->SBUF partition-swap DMAs for all 5 tiles; sin tables sign-folded
    and q tables pre-scaled by 1/sqrt(hd) on the host.  cos-product runs on
    gpsimd, sin-product + final add on DVE; the add writes k+q0..3 straight
    into the per-chunk kqT tile (bf16).
  - Scores transposed, ST[j, i] = kT.T @ qT; exp on ACT (no max-subtraction;
    scores bounded |S|~10); causal handled by computing only j-tiles below
    the diagonal, with per-j-tile narrowed moving operands on the diagonal
    (widths 512/384/256/128) and a single [128,128] upper-tri bf16 mask
    multiply after exp.  Row sums via an all-ones stationary matmul
    accumulated alongside PV; normalize = reciprocal + multiply, writing the
    attention output back into the kqT q-slot (range-based WAR keeps this
    safe and saves SBUF).
  - Stage C accumulates the 4 heads per (i-tile, ech) in PSUM, evicts f32 to
    a packed [128, 4, 512] tile, 2 output DMAs per i-tile.
  - DMA queues: sync = x + half the output, scalar (ACT) = weights + other
    half, vector = rope tables / consts / rotate-swaps.
"""

import math
from contextlib import ExitStack

import numpy as np

import concourse.bass as bass
import concourse.tile as tile
from concourse import bacc, mybir
from concourse.bass_utils import run_bass_kernel_spmd

# ---- problem shape (hardcoded per contract) ----
S = 2048           # seq len
D = 4096           # model dim
HD = 128           # head dim
N_CORES = 8
NQH = 4            # q heads per core
QCOLS = NQH * HD   # 512 wq columns per core
SC_N = 4           # seq chunks of 512
KT_N = D // 128    # 32 contraction tiles
JT_N = S // 128    # 16 key tiles
ECH_N = D // 512   # 8 output column chunks

F32 = mybir.dt.float32
BF16 = mybir.dt.bfloat16

MM_DT = BF16          # matmul operand dtype

_BUILD_CACHE: dict = {}


def _emit_body(nc, tc, dram, out, causal: bool):
    with ExitStack() as ctx:
        def pool(name, bufs, space="SBUF"):
            return ctx.enter_context(tc.tile_pool(name=name, bufs=bufs, space=space))

        wp = pool("w", 1)
        xp = pool("x", 3)
        rpp = pool("rope", 2)
        kqp = pool("kq", 1)
        rtp = pool("rt", 1)
        pep = pool("pe", 3)
        rcp = pool("rc", 2)
        obp = pool("ob", 2)
        vfp = pool("vf", 2)
        ps = ctx.enter_context(tc.tile_pool(name="ps", bufs=1, space="PSUM"))

        # ---- resident weights & constants ----
        wk_sb = wp.tile([128, KT_N, HD], MM_DT, tag="wk", name="wk_sb")
        nc.scalar.dma_start(wk_sb[:], dram["wk"][:])
        wv_sb = wp.tile([128, KT_N, HD], MM_DT, tag="wv", name="wv_sb")
        nc.scalar.dma_start(wv_sb[:], dram["wv"][:])
        wq_sb = wp.tile([128, KT_N, QCOLS], MM_DT, tag="wq", name="wq_sb")
        for i in range(2):
            nc.scalar.dma_start(
                wq_sb[:, i * 16:(i + 1) * 16, :], dram["wq"][:, i * 16:(i + 1) * 16, :]
            )
        ones_sb = wp.tile([128, 128], MM_DT, tag="ones", name="ones_sb")
        nc.scalar.dma_start(ones_sb[:], dram["ones"][:])
        ident_sb = wp.tile([128, 128], MM_DT, tag="ident", name="ident_sb")
        nc.scalar.dma_start(ident_sb[:], dram["ident"][:])
        tri_sb = wp.tile([128, 128], MM_DT, tag="tri", name="tri_sb")
        nc.scalar.dma_start(tri_sb[:], dram["tri"][:])
        wo_sb = wp.tile([128, ECH_N, NQH, 512], MM_DT, tag="wo", name="wo_sb")
        for i in range(2):
            nc.scalar.dma_start(
                wo_sb[:, i * 4:(i + 1) * 4, :, :], dram["wo"][:, i * 4:(i + 1) * 4, :, :]
            )

        # ---- per-chunk persistent activations ----
        # kqT[sc]: t=0 is kT, t=1..4 are qT (overwritten by attention output)
        kqT = [
            kqp.tile([128, 5, 512], MM_DT, tag=f"kq{sc}", name=f"kq{sc}")
            for sc in range(SC_N)
        ]
        v_sb = [
            kqp.tile([128, 4, HD], MM_DT, tag=f"v{sc}", name=f"v{sc}")
            for sc in range(SC_N)
        ]

        def stage_a(sc):
            # x for the chunk in two half-contraction tiles (bufs=3 gives a
            # half-chunk of prefetch ahead of compute)
            xlo = xp.tile([128, 16, 512], MM_DT, tag="xt", name="xlo")
            xhi = xp.tile([128, 16, 512], MM_DT, tag="xt", name="xhi")
            for i in range(4):
                nc.sync.dma_start(
                    xlo[:, i * 4:(i + 1) * 4, :], dram["xn"][sc, i]
                )
            for i in range(4):
                nc.sync.dma_start(
                    xhi[:, i * 4:(i + 1) * 4, :], dram["xn"][sc, 4 + i]
                )

            def xsl(kt):
                return (xlo if kt < 16 else xhi)[:, kt % 16, :]

            rp = rpp.tile([HD, 4, 512], MM_DT, tag="rp", name="rp")
            nc.scalar.dma_start(rp[:], dram["rope"][sc])
            cq, sq, ck, sk = rp[:, 0, :], rp[:, 1, :], rp[:, 2, :], rp[:, 3, :]

            qc = rtp.tile([128, 5, 512], MM_DT, tag="qc", name="qc")
            # out-tile order: k, q0..q3 (rope group), then vT
            for t in range(5):
                pacc = ps.tile([128, 512], F32, tag="psa", name="psa", bufs=2)
                w_ap = wk_sb if t == 0 else wq_sb
                csl = slice(0, HD) if t == 0 else slice((t - 1) * HD, t * HD)
                for kt in range(KT_N):
                    nc.tensor.matmul(
                        pacc[:],
                        w_ap[:, kt, csl],
                        xsl(kt),
                        start=kt == 0,
                        stop=kt == KT_N - 1,
                    )
                nc.vector.tensor_copy(qc[:, t, :], pacc[:])
            # rotate-half partition swap for all 5 tiles at once
            qs = rtp.tile([128, 5, 512], MM_DT, tag="qs", name="qs")
            nc.scalar.dma_start(qs[0:64, :, :], qc[64:128, :, :])
            nc.scalar.dma_start(qs[64:128, :, :], qc[0:64, :, :])
            # sin product (sign-folded tables) on DVE, cos product on gpsimd
            nc.vector.tensor_mul(qs[:, 0, :], qs[:, 0, :], sk)
            nc.gpsimd.tensor_mul(qc[:, 0, :], qc[:, 0, :], ck)
            for h in range(NQH):
                nc.vector.tensor_mul(qs[:, 1 + h, :], qs[:, 1 + h, :], sq)
                nc.gpsimd.tensor_mul(qc[:, 1 + h, :], qc[:, 1 + h, :], cq)
            nc.vector.tensor_add(kqT[sc][:], qc[:], qs[:])

            # vT projection + transpose to v_sb[sc] ([j, d] layout)
            pv = ps.tile([128, 512], F32, tag="psa", name="psav", bufs=2)
            for kt in range(KT_N):
                nc.tensor.matmul(
                    pv[:], wv_sb[:, kt, :], xsl(kt),
                    start=kt == 0, stop=kt == KT_N - 1,
                )
            vt_f = vfp.tile([128, 512], MM_DT, tag="vt", name="vt")
            nc.vector.tensor_copy(vt_f[:], pv[:])
            ptr = ps.tile([128, 4, 128], MM_DT, tag="st", name="ptr", bufs=2)
            for vi in range(4):
                nc.tensor.transpose(
                    ptr[:, vi, :], vt_f[:, vi * 128:(vi + 1) * 128], ident_sb[:]
                )
            nc.vector.tensor_copy(v_sb[sc][:], ptr[:])

        def stage_b(ic):
            njt = 4 * (ic + 1) if causal else JT_N
            for h in range(NQH):
                pv = ps.tile([128, 512], F32, tag="pv", name="pv", bufs=2)
                rs = ps.tile([128, 512], F32, tag="rs", name="rs", bufs=1)
                for jt in range(njt):
                    dt_ = jt - 4 * ic  # diagonal tile index (causal only)
                    off = 128 * dt_ if (causal and dt_ > 0) else 0
                    st = ps.tile([128, 512], F32, tag="st", name="st", bufs=2)
                    nc.tensor.matmul(
                        st[:, off:],
                        kqT[jt // 4][:, 0, (jt % 4) * 128:(jt % 4 + 1) * 128],
                        kqT[ic][:, 1 + h, off:],
                        start=True,
                        stop=True,
                    )
                    pe = pep.tile([128, 512], MM_DT, tag="pe", name="pe")
                    nc.scalar.activation(
                        pe[:, off:], st[:, off:], mybir.ActivationFunctionType.Exp
                    )
                    if causal and dt_ >= 0:
                        nc.vector.tensor_mul(
                            pe[:, off:off + 128], pe[:, off:off + 128], tri_sb[:]
                        )
                    first, last = jt == 0, jt == njt - 1
                    nc.tensor.matmul(
                        pv[:, off:], v_sb[jt // 4][:, jt % 4, :], pe[:, off:],
                        start=first, stop=last,
                    )
                    nc.tensor.matmul(
                        rs[:, off:], ones_sb[:], pe[:, off:],
                        start=first, stop=last,
                    )
                rc = rcp.tile([128, 512], F32, tag="rc", name="rc")
                nc.vector.reciprocal(rc[:], rs[:])
                # attention output overwrites the q slot (range-based WAR)
                nc.vector.tensor_mul(kqT[ic][:, 1 + h, :], pv[:], rc[:])

        def stage_c(ic):
            for itl in range(4):
                it = 4 * ic + itl
                isl = slice(itl * 128, (itl + 1) * 128)
                for eh in range(2):
                    ob = obp.tile([128, 4, 512], F32, tag="ob", name="ob")
                    for e4 in range(4):
                        ech = eh * 4 + e4
                        pc = ps.tile([128, 512], F32, tag="pc", name="pc", bufs=1)
                        for h in range(NQH):
                            nc.tensor.matmul(
                                pc[:],
                                kqT[ic][:, 1 + h, isl],
                                wo_sb[:, ech, h, :],
                                start=h == 0,
                                stop=h == NQH - 1,
                            )
                        nc.vector.tensor_copy(ob[:, e4, :], pc[:])
                    eng = nc.sync if (it + eh) % 2 else nc.scalar
                    eng.dma_start(out[it, eh], ob[:])

        stage_a(0)
        stage_a(1)
        stage_b(0)
        stage_c(0)
        stage_a(2)
        stage_b(1)
        stage_c(1)
        stage_a(3)
        stage_b(2)
        stage_c(2)
        stage_b(3)
        stage_c(3)


def build_nc(causal: bool = True, reps: int = 1):
    nc = bacc.Bacc(
        "TRN2", target_bir_lowering=False, debug=False, num_devices=N_CORES
    )
    dram = {}
    for name, shape, dt in [
        # host-prepermuted layouts: every DMA reads/writes contiguous
        # per-partition runs
        ("xn", [SC_N, 8, 128, 4, 512], MM_DT),
        ("wq", [128, KT_N, QCOLS], MM_DT),
        ("wk", [128, KT_N, HD], MM_DT),
        ("wv", [128, KT_N, HD], MM_DT),
        ("wo", [128, ECH_N, NQH, 512], MM_DT),
        ("rope", [SC_N, HD, 4, 512], MM_DT),
        ("ones", [128, 128], MM_DT),
        ("ident", [128, 128], MM_DT),
        ("tri", [128, 128], MM_DT),
    ]:
        dram[name] = nc.dram_tensor(name, shape, dt, kind="ExternalInput").ap()
    out = nc.dram_tensor("out", [JT_N, 2, 128, 4, 512], F32,
                         kind="ExternalOutput").ap()

    with tile.TileContext(nc) as tc:
        for _ in range(reps):
            _emit_body(nc, tc, dram, out, causal)

    nc.compile()
    return nc


def get_nc(causal: bool = True):
    if causal not in _BUILD_CACHE:
        _BUILD_CACHE[causal] = build_nc(causal)
    return _BUILD_CACHE[causal]


def _mm_np(a):
    return np.ascontiguousarray(a).astype(mybir.dt.np(MM_DT))


def prep_in_maps(x, sincos, wq, wk, wv, wo):
    """Host-side shard + layout prep. Returns list of per-core input dicts.

    All tensors are pre-permuted so that every device DMA moves contiguous
    per-partition runs (device DMA engines are far more efficient that way).
    """
    x = np.asarray(x, np.float32)
    assert x.shape == (1, S, D)
    # xn[sc, xg, p, k4, n] = x[sc*512 + n, (xg*4 + k4)*128 + p]
    xn = _mm_np(
        x[0].reshape(SC_N, 512, 8, 4, 128).transpose(0, 2, 4, 3, 1)
    )

    sincos = np.asarray(sincos, np.float32)
    sin = sincos[:S, :HD]
    cos = sincos[:S, HD:]
    sinT = np.ascontiguousarray(sin.T)
    cosT = np.ascontiguousarray(cos.T)
    sin_sgn = sinT.copy()
    sin_sgn[:64] = -sinT[:64]
    scale = np.float32(1.0 / math.sqrt(HD))
    # rope[sc, d, tbl, n], tbl order: cosq, sinq, cosk, sink
    rope = np.stack(
        [cosT * scale, sin_sgn * scale, cosT, sin_sgn], axis=0
    ).reshape(4, HD, SC_N, 512).transpose(2, 1, 0, 3)
    rope = _mm_np(rope)

    wq = np.asarray(wq, np.float32)
    wk = np.asarray(wk, np.float32)
    wv = np.asarray(wv, np.float32)
    wo = np.asarray(wo, np.float32)

    ones = _mm_np(np.ones((128, 128), np.float32))
    ident = _mm_np(np.eye(128, dtype=np.float32))
    tri = _mm_np(np.triu(np.ones((128, 128), np.float32)))

    in_maps = []
    for c in range(N_CORES):
        wq_c = wq[:, c * QCOLS:(c + 1) * QCOLS]          # [D, 512]
        wk_c = wk[:, c * HD:(c + 1) * HD]                # [D, 128]
        wv_c = wv[:, c * HD:(c + 1) * HD]
        wo_c = wo[c * QCOLS:(c + 1) * QCOLS, :]          # [512, D]
        in_maps.append(
            {
                "xn": xn,
                # wq[p, kt, m] = wq_c[kt*128 + p, m]
                "wq": _mm_np(
                    wq_c.reshape(KT_N, 128, QCOLS).transpose(1, 0, 2)
                ),
                "wk": _mm_np(
                    wk_c.reshape(KT_N, 128, HD).transpose(1, 0, 2)
                ),
                "wv": _mm_np(
                    wv_c.reshape(KT_N, 128, HD).transpose(1, 0, 2)
                ),
                # wo[p, ech, h, n] = wo_c[h*128 + p, ech*512 + n]
                "wo": _mm_np(
                    wo_c.reshape(NQH, 128, ECH_N, 512).transpose(1, 2, 0, 3)
                ),
                "rope": rope,
                "ones": ones,
                "ident": ident,
                "tri": tri,
            }
        )
    return in_maps


def unpermute_out(out_n):
    """out_n [it, eh, p, e4, n] -> out [S, D]."""
    return np.ascontiguousarray(
        out_n.transpose(0, 2, 1, 3, 4).reshape(S, D)
    )


def check_mask(full_causal_mask, start_pos) -> bool:
    """Returns True for causal (tril) mask, False for all-allowed."""
    sp = int(start_pos)
    assert sp == 0, f"kernel specialized for start_pos=0, got {sp}"
    m = np.asarray(full_causal_mask)
    assert m.shape == (1, 1, S, S)
    m = m[0, 0]
    tril = np.tril(np.ones((S, S), dtype=bool))
    if (m == tril).all():
        return True
    if m.all():
        return False
    raise AssertionError("unsupported mask pattern")


def kernel(
    x,
    start_pos,
    sincos,
    full_causal_mask,
    wq,
    wk,
    wv,
    wo,
    cache_k,
    cache_v,
):
    causal = check_mask(full_causal_mask, start_pos)
    # cache_k/cache_v are zero and fully overwritten in the attended region
    # (start_pos=0, seq_len == max_seq_len) — they do not affect the output.
    nc = get_nc(causal)
    in_maps = prep_in_maps(x, sincos, wq, wk, wv, wo)
    res = run_bass_kernel_spmd(nc, in_maps, list(range(N_CORES)))
    acc = res.results[0]["out"].astype(np.float32)
    for c in range(1, N_CORES):
        acc = acc + res.results[c]["out"]
    return unpermute_out(acc)[np.newaxis]


# revision 13
# speedup vs baseline: 3.9816x; 1.1576x over previous
"""Tensor-parallel GQA attention (Llama-3-8B shape, prefill, start_pos=0) on 8
Trainium2 NeuronCores.

Sharding: core i owns kv-head i and q-heads 4i..4i+3 — wq/wk/wv column-shards,
wo row-shard, x replicated.  Each core computes a partial [2048, 4096] output
(its heads pushed through its wo rows); the host sums the 8 partials
(all-reduce equivalent).

v2 layout (all matmul operands bf16, fp32 PSUM accumulate):
  - Stages are interleaved per 512-wide seq chunk: A(sc) projections+RoPE,
    B(ic) attention, C(ic) wo-matmul, emitted A0 A1 B0 C0 A2 B1 C1 A3 B2 C2
    B3 C3 so the tensor engine never drains between stages.
  - Stage A is out-tile-major: x for a whole chunk is SBUF-resident
    ([128, 32, 512] bf16, double-buffered), each of the 6 output tiles
    (k, q0..q3, vT) accumulates its 32 contraction matmuls in a single PSUM
    bank, so stage A holds only 2 PSUM banks and B/C can run concurrently.
  - RoPE: all 5 rotated tiles (k, q0..3) evict into one [128, 5, 512] f32
    SBUF tile (freeing PSUM immediately); rotate-half is one pair of
    SBUF->SBUF partition-swap DMAs for all 5 tiles; sin tables sign-folded
    and q tables pre-scaled by 1/sqrt(hd) on the host.  cos-product runs on
    gpsimd, sin-product + final add on DVE; the add writes k+q0..3 straight
    into the per-chunk kqT tile (bf16).
  - Scores transposed, ST[j, i] = kT.T @ qT; exp on ACT (no max-subtraction;
    scores bounded |S|~10); causal handled by computing only j-tiles below
    the diagonal, with per-j-tile narrowed moving operands on the diagonal
    (widths 512/384/256/128) and a single [128,128] upper-tri bf16 mask
    multiply after exp.  Row sums via an all-ones stationary matmul
    accumulated alongside PV; normalize = reciprocal + multiply, writing the
    attention output back into the kqT q-slot (range-based WAR keeps this
    safe and saves SBUF).
  - Stage C accumulates the 4 heads per (i-tile, ech) in PSUM, evicts f32 to
    a packed [128, 4, 512] tile, 2 output DMAs per i-tile.
  - DMA queues: sync = x + half the output, scalar (ACT) = weights + other
    half, vector = rope tables / consts / rotate-swaps.
"""

import math
from contextlib import ExitStack

import numpy as np

import concourse.bass as bass
import concourse.tile as tile
from concourse import bacc, mybir
from concourse.bass_utils import run_bass_kernel_spmd

# ---- problem shape (hardcoded per contract) ----
S = 2048           # seq len
D = 4096           # model dim
HD = 128           # head dim
N_CORES = 8
NQH = 4            # q heads per core
QCOLS = NQH * HD   # 512 wq columns per core
SC_N = 4           # seq chunks of 512
KT_N = D // 128    # 32 contraction tiles
JT_N = S // 128    # 16 key tiles
ECH_N = D // 512   # 8 output column chunks

F32 = mybir.dt.float32
BF16 = mybir.dt.bfloat16

MM_DT = BF16          # matmul operand dtype

_BUILD_CACHE: dict = {}


def _make_pools(tc, ctx):
    def pool(name, bufs, space="SBUF"):
        return ctx.enter_context(tc.tile_pool(name=name, bufs=bufs, space=space))

    return {
        "wp": pool("w", 1),
        "xp": pool("x", 3),
        "rpp": pool("rope", 2),
        "kqp": pool("kq", 1),
        "rtp": pool("rt", 1),
        "pep": pool("pe", 3),
        "rcp": pool("rc", 2),
        "obp": pool("ob", 2),
        "vfp": pool("vf", 2),
        "ps": pool("ps", 1, space="PSUM"),
    }


def _emit_body(nc, pools, dram, out, causal: bool):
    if True:
        wp = pools["wp"]
        xp = pools["xp"]
        rpp = pools["rpp"]
        kqp = pools["kqp"]
        rtp = pools["rtp"]
        pep = pools["pep"]
        rcp = pools["rcp"]
        obp = pools["obp"]
        vfp = pools["vfp"]
        ps = pools["ps"]

        # ---- resident weights & constants ----
        wk_sb = wp.tile([128, KT_N, HD], MM_DT, tag="wk", name="wk_sb")
        nc.scalar.dma_start(wk_sb[:], dram["wk"][:])
        wv_sb = wp.tile([128, KT_N, HD], MM_DT, tag="wv", name="wv_sb")
        nc.scalar.dma_start(wv_sb[:], dram["wv"][:])
        wq_sb = wp.tile([128, KT_N, QCOLS], MM_DT, tag="wq", name="wq_sb")
        for i in range(2):
            nc.scalar.dma_start(
                wq_sb[:, i * 16:(i + 1) * 16, :], dram["wq"][:, i * 16:(i + 1) * 16, :]
            )
        ones_sb = wp.tile([128, 128], MM_DT, tag="ones", name="ones_sb")
        nc.scalar.dma_start(ones_sb[:], dram["ones"][:])
        ident_sb = wp.tile([128, 128], MM_DT, tag="ident", name="ident_sb")
        nc.scalar.dma_start(ident_sb[:], dram["ident"][:])
        tri_sb = wp.tile([128, 128], MM_DT, tag="tri", name="tri_sb")
        nc.scalar.dma_start(tri_sb[:], dram["tri"][:])
        wo_sb = wp.tile([128, ECH_N, NQH, 512], MM_DT, tag="wo", name="wo_sb")
        for i in range(2):
            nc.scalar.dma_start(
                wo_sb[:, i * 4:(i + 1) * 4, :, :], dram["wo"][:, i * 4:(i + 1) * 4, :, :]
            )

        # ---- per-chunk persistent activations ----
        # kqT[sc]: t=0 is kT, t=1..4 are qT (overwritten by attention output)
        kqT = [
            kqp.tile([128, 5, 512], MM_DT, tag=f"kq{sc}", name=f"kq{sc}")
            for sc in range(SC_N)
        ]
        v_sb = [
            kqp.tile([128, 4, HD], MM_DT, tag=f"v{sc}", name=f"v{sc}")
            for sc in range(SC_N)
        ]

        def stage_a(sc):
            # x for the chunk in two half-contraction tiles (bufs=3 gives a
            # half-chunk of prefetch ahead of compute)
            xlo = xp.tile([128, 16, 512], MM_DT, tag="xt", name="xlo")
            xhi = xp.tile([128, 16, 512], MM_DT, tag="xt", name="xhi")
            for i in range(4):
                nc.sync.dma_start(
                    xlo[:, i * 4:(i + 1) * 4, :], dram["xn"][sc, i]
                )
            for i in range(4):
                nc.sync.dma_start(
                    xhi[:, i * 4:(i + 1) * 4, :], dram["xn"][sc, 4 + i]
                )

            def xsl(kt):
                return (xlo if kt < 16 else xhi)[:, kt % 16, :]

            rp = rpp.tile([HD, 4, 512], MM_DT, tag="rp", name="rp")
            nc.scalar.dma_start(rp[:], dram["rope"][sc])
            cq, sq, ck, sk = rp[:, 0, :], rp[:, 1, :], rp[:, 2, :], rp[:, 3, :]

            qc = rtp.tile([128, 5, 512], MM_DT, tag="qc", name="qc")
            # out-tile order: k, q0..q3 (rope group), then vT
            for t in range(5):
                pacc = ps.tile([128, 512], F32, tag="psa", name="psa", bufs=2)
                w_ap = wk_sb if t == 0 else wq_sb
                csl = slice(0, HD) if t == 0 else slice((t - 1) * HD, t * HD)
                for kt in range(KT_N):
                    nc.tensor.matmul(
                        pacc[:],
                        w_ap[:, kt, csl],
                        xsl(kt),
                        start=kt == 0,
                        stop=kt == KT_N - 1,
                    )
                nc.vector.tensor_copy(qc[:, t, :], pacc[:])
            # rotate-half partition swap for all 5 tiles at once
            qs = rtp.tile([128, 5, 512], MM_DT, tag="qs", name="qs")
            nc.scalar.dma_start(qs[0:64, :, :], qc[64:128, :, :])
            nc.scalar.dma_start(qs[64:128, :, :], qc[0:64, :, :])
            # sin product (sign-folded tables) on DVE, cos product on gpsimd
            nc.vector.tensor_mul(qs[:, 0, :], qs[:, 0, :], sk)
            nc.gpsimd.tensor_mul(qc[:, 0, :], qc[:, 0, :], ck)
            for h in range(NQH):
                nc.vector.tensor_mul(qs[:, 1 + h, :], qs[:, 1 + h, :], sq)
                nc.gpsimd.tensor_mul(qc[:, 1 + h, :], qc[:, 1 + h, :], cq)
            nc.vector.tensor_add(kqT[sc][:], qc[:], qs[:])

            # vT projection + transpose to v_sb[sc] ([j, d] layout)
            pv = ps.tile([128, 512], F32, tag="psa", name="psav", bufs=2)
            for kt in range(KT_N):
                nc.tensor.matmul(
                    pv[:], wv_sb[:, kt, :], xsl(kt),
                    start=kt == 0, stop=kt == KT_N - 1,
                )
            vt_f = vfp.tile([128, 512], MM_DT, tag="vt", name="vt")
            nc.vector.tensor_copy(vt_f[:], pv[:])
            ptr = ps.tile([128, 4, 128], MM_DT, tag="st", name="ptr", bufs=2)
            for vi in range(4):
                nc.tensor.transpose(
                    ptr[:, vi, :], vt_f[:, vi * 128:(vi + 1) * 128], ident_sb[:]
                )
            nc.vector.tensor_copy(v_sb[sc][:], ptr[:])

        def stage_b(ic):
            njt = 4 * (ic + 1) if causal else JT_N
            for h in range(NQH):
                pv = ps.tile([128, 512], F32, tag="pv", name="pv", bufs=2)
                rs = ps.tile([128, 512], F32, tag="rs", name="rs", bufs=1)
                for jt in range(njt):
                    dt_ = jt - 4 * ic  # diagonal tile index (causal only)
                    off = 128 * dt_ if (causal and dt_ > 0) else 0
                    st = ps.tile([128, 512], F32, tag="st", name="st", bufs=2)
                    nc.tensor.matmul(
                        st[:, off:],
                        kqT[jt // 4][:, 0, (jt % 4) * 128:(jt % 4 + 1) * 128],
                        kqT[ic][:, 1 + h, off:],
                        start=True,
                        stop=True,
                    )
                    pe = pep.tile([128, 512], MM_DT, tag="pe", name="pe")
                    nc.scalar.activation(
                        pe[:, off:], st[:, off:], mybir.ActivationFunctionType.Exp
                    )
                    if causal and dt_ >= 0:
                        nc.vector.tensor_mul(
                            pe[:, off:off + 128], pe[:, off:off + 128], tri_sb[:]
                        )
                    first, last = jt == 0, jt == njt - 1
                    nc.tensor.matmul(
                        pv[:, off:], v_sb[jt // 4][:, jt % 4, :], pe[:, off:],
                        start=first, stop=last,
                    )
                    nc.tensor.matmul(
                        rs[:, off:], ones_sb[:], pe[:, off:],
                        start=first, stop=last,
                    )
                rc = rcp.tile([128, 512], F32, tag="rc", name="rc")
                nc.vector.reciprocal(rc[:], rs[:])
                # attention output overwrites the q slot (range-based WAR)
                nc.vector.tensor_mul(kqT[ic][:, 1 + h, :], pv[:], rc[:])

        def stage_c(ic):
            for itl in range(4):
                it = 4 * ic + itl
                isl = slice(itl * 128, (itl + 1) * 128)
                for eh in range(2):
                    ob = obp.tile([128, 4, 512], MM_DT, tag="ob", name="ob")
                    for e4 in range(4):
                        ech = eh * 4 + e4
                        pc = ps.tile([128, 512], F32, tag="pc", name="pc", bufs=1)
                        for h in range(NQH):
                            nc.tensor.matmul(
                                pc[:],
                                kqT[ic][:, 1 + h, isl],
                                wo_sb[:, ech, h, :],
                                start=h == 0,
                                stop=h == NQH - 1,
                            )
                        nc.vector.tensor_copy(ob[:, e4, :], pc[:])
                    eng = nc.sync if (it + eh) % 2 else nc.scalar
                    eng.dma_start(out[it, eh], ob[:])

        stage_a(0)
        stage_a(1)
        stage_b(0)
        stage_c(0)
        stage_a(2)
        stage_b(1)
        stage_c(1)
        stage_a(3)
        stage_b(2)
        stage_c(2)
        stage_b(3)
        stage_c(3)


def build_nc(causal: bool = True, reps: int = 1):
    nc = bacc.Bacc(
        "TRN2", target_bir_lowering=False, debug=False, num_devices=N_CORES
    )
    dram = {}
    for name, shape, dt in [
        # host-prepermuted layouts: every DMA reads/writes contiguous
        # per-partition runs
        ("xn", [SC_N, 8, 128, 4, 512], MM_DT),
        ("wq", [128, KT_N, QCOLS], MM_DT),
        ("wk", [128, KT_N, HD], MM_DT),
        ("wv", [128, KT_N, HD], MM_DT),
        ("wo", [128, ECH_N, NQH, 512], MM_DT),
        ("rope", [SC_N, HD, 4, 512], MM_DT),
        ("ones", [128, 128], MM_DT),
        ("ident", [128, 128], MM_DT),
        ("tri", [128, 128], MM_DT),
    ]:
        dram[name] = nc.dram_tensor(name, shape, dt, kind="ExternalInput").ap()
    out = nc.dram_tensor("out", [JT_N, 2, 128, 4, 512], MM_DT,
                         kind="ExternalOutput").ap()

    with tile.TileContext(nc) as tc:
        with ExitStack() as ctx:
            pools = _make_pools(tc, ctx)
            for _ in range(reps):
                _emit_body(nc, pools, dram, out, causal)

    nc.compile()
    return nc


def get_nc(causal: bool = True):
    if causal not in _BUILD_CACHE:
        _BUILD_CACHE[causal] = build_nc(causal)
    return _BUILD_CACHE[causal]


def _mm_np(a):
    return np.ascontiguousarray(a).astype(mybir.dt.np(MM_DT))


def prep_in_maps(x, sincos, wq, wk, wv, wo):
    """Host-side shard + layout prep. Returns list of per-core input dicts.

    All tensors are pre-permuted so that every device DMA moves contiguous
    per-partition runs (device DMA engines are far more efficient that way).
    """
    x = np.asarray(x, np.float32)
    assert x.shape == (1, S, D)
    # xn[sc, xg, p, k4, n] = x[sc*512 + n, (xg*4 + k4)*128 + p]
    xn = _mm_np(
        x[0].reshape(SC_N, 512, 8, 4, 128).transpose(0, 2, 4, 3, 1)
    )

    sincos = np.asarray(sincos, np.float32)
    sin = sincos[:S, :HD]
    cos = sincos[:S, HD:]
    sinT = np.ascontiguousarray(sin.T)
    cosT = np.ascontiguousarray(cos.T)
    sin_sgn = sinT.copy()
    sin_sgn[:64] = -sinT[:64]
    scale = np.float32(1.0 / math.sqrt(HD))
    # rope[sc, d, tbl, n], tbl order: cosq, sinq, cosk, sink
    rope = np.stack(
        [cosT * scale, sin_sgn * scale, cosT, sin_sgn], axis=0
    ).reshape(4, HD, SC_N, 512).transpose(2, 1, 0, 3)
    rope = _mm_np(rope)

    wq = np.asarray(wq, np.float32)
    wk = np.asarray(wk, np.float32)
    wv = np.asarray(wv, np.float32)
    wo = np.asarray(wo, np.float32)

    ones = _mm_np(np.ones((128, 128), np.float32))
    ident = _mm_np(np.eye(128, dtype=np.float32))
    tri = _mm_np(np.triu(np.ones((128, 128), np.float32)))

    in_maps = []
    for c in range(N_CORES):
        wq_c = wq[:, c * QCOLS:(c + 1) * QCOLS]          # [D, 512]
        wk_c = wk[:, c * HD:(c + 1) * HD]                # [D, 128]
        wv_c = wv[:, c * HD:(c + 1) * HD]
        wo_c = wo[c * QCOLS:(c + 1) * QCOLS, :]          # [512, D]
        in_maps.append(
            {
                "xn": xn,
                # wq[p, kt, m] = wq_c[kt*128 + p, m]
                "wq": _mm_np(
                    wq_c.reshape(KT_N, 128, QCOLS).transpose(1, 0, 2)
                ),
                "wk": _mm_np(
                    wk_c.reshape(KT_N, 128, HD).transpose(1, 0, 2)
                ),
                "wv": _mm_np(
                    wv_c.reshape(KT_N, 128, HD).transpose(1, 0, 2)
                ),
                # wo[p, ech, h, n] = wo_c[h*128 + p, ech*512 + n]
                "wo": _mm_np(
                    wo_c.reshape(NQH, 128, ECH_N, 512).transpose(1, 2, 0, 3)
                ),
                "rope": rope,
                "ones": ones,
                "ident": ident,
                "tri": tri,
            }
        )
    return in_maps


def unpermute_out(out_n):
    """out_n [it, eh, p, e4, n] -> out [S, D]."""
    return np.ascontiguousarray(
        out_n.transpose(0, 2, 1, 3, 4).reshape(S, D)
    )


def check_mask(full_causal_mask, start_pos) -> bool:
    """Returns True for causal (tril) mask, False for all-allowed."""
    sp = int(start_pos)
    assert sp == 0, f"kernel specialized for start_pos=0, got {sp}"
    m = np.asarray(full_causal_mask)
    assert m.shape == (1, 1, S, S)
    m = m[0, 0]
    tril = np.tril(np.ones((S, S), dtype=bool))
    if (m == tril).all():
        return True
    if m.all():
        return False
    raise AssertionError("unsupported mask pattern")


def kernel(
    x,
    start_pos,
    sincos,
    full_causal_mask,
    wq,
    wk,
    wv,
    wo,
    cache_k,
    cache_v,
):
    causal = check_mask(full_causal_mask, start_pos)
    # cache_k/cache_v are zero and fully overwritten in the attended region
    # (start_pos=0, seq_len == max_seq_len) — they do not affect the output.
    nc = get_nc(causal)
    in_maps = prep_in_maps(x, sincos, wq, wk, wv, wo)
    res = run_bass_kernel_spmd(nc, in_maps, list(range(N_CORES)))
    acc = res.results[0]["out"].astype(np.float32)
    for c in range(1, N_CORES):
        acc = acc + res.results[c]["out"].astype(np.float32)
    return unpermute_out(acc)[np.newaxis]
